# revision 1
# baseline (speedup 1.0000x reference)
"""Trainium2 Bass kernel for GPT-2 style attention block (B=2, S=2048, D=1024, H=16).

Sharding (8 cores): data-parallel over batch (2) x tensor-parallel over heads (4 per
core). Each core: QKV projection for its 4 heads over the full sequence, full-seq
causal attention (transposed-scores layout: softmax reduction folded into the PV
matmul via a ones-column in V), then an AllToAll over all 8 cores (mesh needs >4
per group; shards replicated into both batch halves, receivers select their batch
via a 0/1 input) converts head-sharding to sequence-sharding so c_proj runs
reduction-free. Host only concatenates the per-core outputs.

Compute dtype bf16 (fp32 PSUM accumulation); masks/normalization in fp32.
"""
import sys
sys.path.insert(0, '/opt/trn_rl_repo')

import numpy as np
import ml_dtypes

import concourse.bass as bass
import concourse.mybir as mybir
import concourse.tile as tile
from concourse import bacc
from concourse.bass_utils import run_bass_kernel_spmd

B, S, D = 2, 2048, 1024
H, HD = 16, 64
NCORES = 8
HPC = H // 4          # heads per core = 4

F32 = mybir.dt.float32
BF16 = mybir.dt.bfloat16
ADD = mybir.AluOpType.add
MULT = mybir.AluOpType.mult
BYPASS = mybir.AluOpType.bypass
EXP = mybir.ActivationFunctionType.Exp


def _emit(nc, tc):
    xT = nc.dram_tensor("xT", [D, S], BF16, kind="ExternalInput").ap()
    w_qk = nc.dram_tensor("w_qk", [D, 512], BF16, kind="ExternalInput").ap()
    w_v = nc.dram_tensor("w_v", [D, 256], BF16, kind="ExternalInput").ap()
    w_p = nc.dram_tensor("w_p", [D, D], BF16, kind="ExternalInput").ap()
    bqk = nc.dram_tensor("bqk", [128, 4], F32, kind="ExternalInput").ap()
    zsel = nc.dram_tensor("zsel", [128, 2], F32, kind="ExternalInput").ap()
    beff = nc.dram_tensor("beff", [128, D], F32, kind="ExternalInput").ap()
    cmask = nc.dram_tensor("cmask", [128, 128], F32, kind="ExternalInput").ap()
    out = nc.dram_tensor("out", [512, D], F32, kind="ExternalOutput").ap()

    a2a_in = [nc.dram_tensor(f"a2a_in{u}", [8, 128, 512], BF16) for u in range(2)]
    a2a_out = [nc.dram_tensor(f"a2a_out{u}", [8, 128, 512], BF16) for u in range(2)]

    from contextlib import ExitStack
    ctx = ExitStack()
    cst = ctx.enter_context(tc.tile_pool(name="cst", bufs=1))
    pw = ctx.enter_context(tc.tile_pool(name="pw", bufs=4, space="PSUM"))
    psc = ctx.enter_context(tc.tile_pool(name="psc", bufs=2, space="PSUM"))
    sb = ctx.enter_context(tc.tile_pool(name="sb", bufs=3))

    # ---- resident SBUF loads (split per k-subtile so PE can start early) ----
    xT_sb = cst.tile([128, 8, S], BF16)
    wqk_sb = cst.tile([128, 8, 512], BF16)
    wv_sb = cst.tile([128, 8, 256], BF16)
    for k in range(8):
        nc.sync.dma_start(xT_sb[:, k], xT.rearrange("(k p) n -> p k n", p=128)[:, k])
        nc.sync.dma_start(wqk_sb[:, k], w_qk.rearrange("(k p) n -> p k n", p=128)[:, k])
        nc.sync.dma_start(wv_sb[:, k], w_v.rearrange("(k p) n -> p k n", p=128)[:, k])
    wp_sb = cst.tile([128, 8, D], BF16)
    nc.sync.dma_start(wp_sb[:], w_p.rearrange("(k p) n -> p k n", p=128))
    bqk_sb = cst.tile([128, 4], F32)
    nc.sync.dma_start(bqk_sb[:], bqk)
    zsel_sb = cst.tile([128, 2], F32)
    nc.sync.dma_start(zsel_sb[:], zsel)
    beff_sb = cst.tile([128, D], F32)
    nc.sync.dma_start(beff_sb[:], beff)
    cm_sb = cst.tile([128, 128], F32)
    nc.sync.dma_start(cm_sb[:], cmask)
    ones_sb = cst.tile([1, 64], BF16)
    nc.vector.memset(ones_sb[:], 1.0)

    # PE warmer: dependency-free junk matmuls keep the array busy during the
    # input DMAs so HAM unthrottles before real work arrives
    wrow = sb.tile([1, 512], BF16, tag="wrow")
    nc.vector.memset(wrow[:], 1.0)
    warm_ps = pw.tile([128, 512], F32, tag="w", name="warm")
    for _ in range(40):
        nc.tensor.matmul(warm_ps[0:64, :], ones_sb[:], wrow[:],
                         start=True, stop=True)

    # qkT [512, 2048]: rows 0-255 = q^T (4 heads x 64, prescaled 1/8), 256-511 = k^T
    qkT_sb = cst.tile([128, 4, S], BF16)

    def qk_proj(m, qts=(0, 1, 2, 3)):
        ps = {qt: pw.tile([128, 512], F32, tag="w", name=f"qk{m}_{qt}") for qt in qts}
        for k in range(8):
            for qt in qts:
                nc.tensor.matmul(
                    ps[qt][:], wqk_sb[:, k, m * 128:(m + 1) * 128],
                    xT_sb[:, k, qt * 512:(qt + 1) * 512],
                    start=(k == 0), stop=(k == 7))
        for qt in qts:
            nc.vector.tensor_scalar(
                out=qkT_sb[:, m, qt * 512:(qt + 1) * 512], in0=ps[qt][:],
                scalar1=bqk_sb[:, m:m + 1], scalar2=None, op0=ADD)

    # V with interleaved ones column: V_sb [128, 16, 4*65]
    V_sb = cst.tile([128, 16, HPC * 65], BF16)

    def v_ones():
        nc.vector.memset(
            V_sb[:].rearrange("p m (h c) -> p m h c", c=65)[:, :, :, 64:65], 1.0)

    def v_piece(m):
        ps = pw.tile([128, 512], F32, tag="w", name=f"v{m}")
        for k in range(8):
            nc.tensor.matmul(
                ps[:, :256], xT_sb[:, k, m * 128:(m + 1) * 128], wv_sb[:, k, :],
                start=(k == 0), stop=(k == 7))
        nc.vector.tensor_copy(
            out=V_sb[:, m].rearrange("p (h c) -> p h c", c=65)[:, :, 0:64],
            in_=ps[:, :256].rearrange("p (h c) -> p h c", c=64))

    attnT_sb = cst.tile([128, 2, S], BF16)
    proj_sb = cst.tile([128, 8, 512], BF16)  # gathered attnT for my 512 q rows

    def attend_qt(h, qt, fillers=None):
        if True:
            sub, po = h // 2, 64 * (h % 2)
            at = pw.tile([128, 512], F32, tag="w", name=f"at{h}_{qt}")
            nkb = 4 * qt + 4
            for g0 in range(0, nkb, 2):
                gl = list(range(g0, min(g0 + 2, nkb)))
                sc = psc.tile([128, 1024], F32, tag="sc")
                for i, kb in enumerate(gl):
                    rel = max(0, kb * 128 - qt * 512)
                    nc.tensor.matmul(
                        sc[:, i * 512:(i + 1) * 512],
                        qkT_sb[po:po + 64, 2 + sub, kb * 128:(kb + 1) * 128],
                        qkT_sb[po:po + 64, sub, qt * 512:(qt + 1) * 512],
                        start=True, stop=True)
                    if kb * 128 >= qt * 512:  # diagonal 128x128 triangle mask
                        nc.vector.tensor_tensor(
                            sc[:, i * 512 + rel:i * 512 + rel + 128],
                            sc[:, i * 512 + rel:i * 512 + rel + 128],
                            cm_sb[:], ADD)
                pt = sb.tile([128, 1024], BF16, tag="pt")
                w = len(gl) * 512
                nc.scalar.activation(out=pt[:, :w], in_=sc[:, :w], func=EXP)
                for i, kb in enumerate(gl):
                    rel = max(0, kb * 128 - qt * 512)
                    nc.tensor.matmul(
                        at[0:65, rel:512], V_sb[:, kb, h * 65:(h + 1) * 65],
                        pt[:, i * 512 + rel:(i + 1) * 512],
                        start=(kb == 0), stop=(kb == nkb - 1))
            # stash unnormalized attn, take 1/denominator, broadcast, normalize
            sl = attnT_sb[po:po + 64, sub, qt * 512:(qt + 1) * 512]
            nc.vector.tensor_copy(out=sl, in_=at[0:64, :])
            den1 = sb.tile([1, 512], F32, tag="den1")
            nc.vector.tensor_copy(out=den1[:], in_=at[64:65, :])
            rec1 = sb.tile([1, 512], F32, tag="rec1")
            nc.vector.reciprocal_approx_fast(rec1[:], den1[:])
            rec1b = sb.tile([1, 512], BF16, tag="rec1b")
            nc.vector.tensor_copy(out=rec1b[:], in_=rec1[:])
            bc = pw.tile([128, 512], F32, tag="w", name=f"bc{h}_{qt}")
            nc.tensor.matmul(
                bc[0:64, :], ones_sb[:], rec1b[:], start=True, stop=True)
            nc.vector.tensor_tensor(sl, sl, bc[0:64, :], MULT)

    def a2a_send(u):
        for g in range(4):
            src = attnT_sb[:, u, g * 512:(g + 1) * 512]
            nc.sync.dma_start(a2a_in[u].ap()[g], src)
            nc.sync.dma_start(a2a_in[u].ap()[4 + g], src)
        nc.gpsimd.collective_compute(
            "AllToAll", BYPASS, replica_groups=[list(range(NCORES))],
            ins=[a2a_in[u].ap().opt()], outs=[a2a_out[u].ap().opt()])

    def a2a_recv(u):
        for g in range(4):
            t0 = sb.tile([128, 512], BF16, tag="t0")
            t1 = sb.tile([128, 512], BF16, tag="t1")
            nc.sync.dma_start(t0[:], a2a_out[u].ap()[g])
            nc.sync.dma_start(t1[:], a2a_out[u].ap()[4 + g])
            tz = sb.tile([128, 512], BF16, tag="tz")
            nc.vector.tensor_scalar(
                out=tz[:], in0=t0[:], scalar1=zsel_sb[:, 0:1], scalar2=None, op0=MULT)
            nc.vector.scalar_tensor_tensor(
                out=proj_sb[:, 2 * g + u, :], in0=t1[:], scalar=zsel_sb[:, 1:2],
                in1=tz[:], op0=MULT, op1=ADD)

    def c_proj(ms):
        korder = [0, 2, 4, 6, 1, 3, 5, 7]  # chunk-0 subtiles first
        for m in ms:
            out_sb = sb.tile([128, D], F32, tag="out")
            ps = [pw.tile([128, 512], F32, tag="w", name=f"pj{m}_{n}") for n in range(2)]
            for ki, k in enumerate(korder):
                for n in range(2):
                    nc.tensor.matmul(
                        ps[n][:], proj_sb[:, k, m * 128:(m + 1) * 128],
                        wp_sb[:, k, n * 512:(n + 1) * 512],
                        start=(ki == 0), stop=(ki == 7))
            for n in range(2):
                nc.vector.tensor_tensor(
                    out_sb[:, n * 512:(n + 1) * 512], ps[n][:],
                    beff_sb[:, n * 512:(n + 1) * 512], ADD)
            nc.sync.dma_start(out[m * 128:(m + 1) * 128, :], out_sb[:])

    # ---- emission order tuned for overlap ----
    # emission order: attention pieces interleaved with projection pieces so
    # the PE stream stays dense (HAM stays unthrottled) while ACT chews exps
    qk_proj(0)
    qk_proj(2)
    v_ones()
    for m in range(16):
        v_piece(m)
    for qt in range(4):
        attend_qt(0, qt)
    for qt in range(4):
        attend_qt(1, qt)
    a2a_send(0)
    qk_proj(1)
    qk_proj(3)
    for qt in range(4):
        attend_qt(2, qt)
    a2a_recv(0)
    for qt in range(4):
        attend_qt(3, qt)
    a2a_send(1)
    a2a_recv(1)
    c_proj((0, 1, 2, 3))

    ctx.close()


def build_nc():
    nc = bacc.Bacc("TRN2", target_bir_lowering=False, debug=False, num_devices=NCORES)
    with tile.TileContext(nc) as tc:
        _emit(nc, tc)
    nc.compile()
    return nc


def shard_inputs(hidden_states, c_attn_w, c_attn_b, c_proj_w, c_proj_b):
    x = np.asarray(hidden_states, np.float32)
    W = np.asarray(c_attn_w, np.float32)
    bqkv = np.asarray(c_attn_b, np.float32)
    Wp = np.asarray(c_proj_w, np.float32)
    bp = np.asarray(c_proj_b, np.float32)

    wq, wk, wv = W[:, :D] * 0.125, W[:, D:2 * D], W[:, 2 * D:]
    bq, bk, bv = bqkv[:D] * 0.125, bqkv[D:2 * D], bqkv[2 * D:]
    beff = np.broadcast_to(bp + bv @ Wp, (128, D)).astype(np.float32).copy()
    wp_bf = Wp.astype(ml_dtypes.bfloat16)

    # 128x128 causal triangle: -1e4 where key (row) > query (col)
    k_i = np.arange(128)[:, None]
    q_i = np.arange(128)[None, :]
    cm = np.where(k_i > q_i, np.float32(-10000.0), np.float32(0.0)).astype(np.float32)

    in_maps = []
    for c in range(NCORES):
        b, r = divmod(c, 4)
        hs = slice(256 * r, 256 * (r + 1))
        w_qk = np.concatenate([wq[:, hs], wk[:, hs]], axis=1)
        bqk_t = np.concatenate([bq[hs], bk[hs]]).reshape(4, 128).T.copy()
        zs = np.zeros((128, 2), np.float32)
        zs[:, b] = 1.0
        in_maps.append(dict(
            zsel=zs,
            xT=np.ascontiguousarray(x[b].T).astype(ml_dtypes.bfloat16),
            w_qk=w_qk.astype(ml_dtypes.bfloat16),
            w_v=wv[:, hs].astype(ml_dtypes.bfloat16),
            w_p=wp_bf,
            bqk=bqk_t.astype(np.float32),
            beff=beff,
            cmask=cm,
        ))
    return in_maps


def unshard(results):
    full = np.zeros((B, S, D), np.float32)
    for c in range(NCORES):
        b, r = divmod(c, 4)
        full[b, 512 * r:512 * (r + 1)] = results[c]["out"]
    return full


_NC = None


def kernel(**inputs):
    global _NC
    if _NC is None:
        _NC = build_nc()
    in_maps = shard_inputs(**inputs)
    res = run_bass_kernel_spmd(_NC, in_maps, core_ids=list(range(NCORES)))
    return unshard(res.results)


if __name__ == "__main__":
    import jax
    with jax.default_device(jax.devices("cpu")[0]):
        import reference
        inputs = {k: np.asarray(v) for k, v in reference.setup_inputs().items()}
        expected = np.asarray(reference.reference(**inputs))
    actual = kernel(**inputs)
    err = np.abs(actual - expected)
    print("max abs err:", err.max(), "rel:", err.max() / np.abs(expected).max())



# revision 2
# speedup vs baseline: 1.4879x; 1.4879x over previous
"""Trainium2 Bass kernel for GPT-2 style attention block (B=2, S=2048, D=1024, H=16).

Sharding (8 cores): data-parallel over batch (2) x tensor-parallel over heads (4 per
core). Each core: QKV projection for its 4 heads over the full sequence, full-seq
causal attention (transposed-scores layout: softmax reduction folded into the PV
matmul via a ones-column in V), then a row-parallel partial c_proj over the full
sequence using only this core's 256 rows of c_proj_w. No collectives: the host
sums the 4 per-head-group partials per batch (plus the folded v-bias term), so
each core's span is pure compute with no cross-core sync.

Compute dtype bf16 (fp32 PSUM accumulation); masks/normalization in fp32;
partial outputs shipped as fp16 to halve DMA.
"""
import sys
sys.path.insert(0, '/opt/trn_rl_repo')

import numpy as np
import ml_dtypes

import concourse.bass as bass
import concourse.mybir as mybir
import concourse.tile as tile
from concourse import bacc
from concourse.bass_utils import run_bass_kernel_spmd

B, S, D = 2, 2048, 1024
H, HD = 16, 64
NCORES = 8
HPC = H // 4          # heads per core = 4

F32 = mybir.dt.float32
F16 = mybir.dt.float16
BF16 = mybir.dt.bfloat16
ADD = mybir.AluOpType.add
MULT = mybir.AluOpType.mult
EXP = mybir.ActivationFunctionType.Exp


def _emit(nc, tc):
    xT = nc.dram_tensor("xT", [D, S], BF16, kind="ExternalInput").ap()
    w_qk = nc.dram_tensor("w_qk", [D, 512], BF16, kind="ExternalInput").ap()
    w_v = nc.dram_tensor("w_v", [D, 256], BF16, kind="ExternalInput").ap()
    w_p = nc.dram_tensor("w_p", [256, D], BF16, kind="ExternalInput").ap()
    bqk = nc.dram_tensor("bqk", [128, 4], F32, kind="ExternalInput").ap()
    cmask = nc.dram_tensor("cmask", [128, 128], F32, kind="ExternalInput").ap()
    out = nc.dram_tensor("out", [S, D], F16, kind="ExternalOutput").ap()

    from contextlib import ExitStack
    ctx = ExitStack()
    cst = ctx.enter_context(tc.tile_pool(name="cst", bufs=1))
    pw = ctx.enter_context(tc.tile_pool(name="pw", bufs=4, space="PSUM"))
    psc = ctx.enter_context(tc.tile_pool(name="psc", bufs=2, space="PSUM"))
    sb = ctx.enter_context(tc.tile_pool(name="sb", bufs=3))

    # ---- resident SBUF loads (split per k-subtile so PE can start early) ----
    xT_sb = cst.tile([128, 8, S], BF16)
    wqk_sb = cst.tile([128, 8, 512], BF16)
    wv_sb = cst.tile([128, 8, 256], BF16)
    for k in range(8):
        nc.sync.dma_start(xT_sb[:, k], xT.rearrange("(k p) n -> p k n", p=128)[:, k])
        nc.sync.dma_start(wqk_sb[:, k], w_qk.rearrange("(k p) n -> p k n", p=128)[:, k])
        nc.sync.dma_start(wv_sb[:, k], w_v.rearrange("(k p) n -> p k n", p=128)[:, k])
    wp_sb = cst.tile([128, 2, D], BF16)
    nc.sync.dma_start(wp_sb[:], w_p.rearrange("(k p) n -> p k n", p=128))
    bqk_sb = cst.tile([128, 4], F32)
    nc.sync.dma_start(bqk_sb[:], bqk)
    cm_sb = cst.tile([128, 128], F32)
    nc.sync.dma_start(cm_sb[:], cmask)
    ones_sb = cst.tile([1, 64], BF16)
    nc.vector.memset(ones_sb[:], 1.0)

    # PE warmer: dependency-free junk matmuls keep the array busy during the
    # input DMAs so HAM unthrottles before real work arrives
    wrow = sb.tile([1, 512], BF16, tag="wrow")
    nc.vector.memset(wrow[:], 1.0)
    warm_ps = pw.tile([128, 512], F32, tag="w", name="warm")
    for _ in range(40):
        nc.tensor.matmul(warm_ps[0:64, :], ones_sb[:], wrow[:],
                         start=True, stop=True)

    # qkT [512, 2048]: rows 0-255 = q^T (4 heads x 64, prescaled 1/8), 256-511 = k^T
    qkT_sb = cst.tile([128, 4, S], BF16)

    def qk_proj(m, qts=(0, 1, 2, 3)):
        ps = {qt: pw.tile([128, 512], F32, tag="w", name=f"qk{m}_{qt}") for qt in qts}
        for k in range(8):
            for qt in qts:
                nc.tensor.matmul(
                    ps[qt][:], wqk_sb[:, k, m * 128:(m + 1) * 128],
                    xT_sb[:, k, qt * 512:(qt + 1) * 512],
                    start=(k == 0), stop=(k == 7))
        for qt in qts:
            nc.vector.tensor_scalar(
                out=qkT_sb[:, m, qt * 512:(qt + 1) * 512], in0=ps[qt][:],
                scalar1=bqk_sb[:, m:m + 1], scalar2=None, op0=ADD)

    # V with interleaved ones column: V_sb [128, 16, 4*65]
    V_sb = cst.tile([128, 16, HPC * 65], BF16)

    def v_ones():
        nc.vector.memset(
            V_sb[:].rearrange("p m (h c) -> p m h c", c=65)[:, :, :, 64:65], 1.0)

    def v_piece(m):
        ps = pw.tile([128, 512], F32, tag="w", name=f"v{m}")
        for k in range(8):
            nc.tensor.matmul(
                ps[:, :256], xT_sb[:, k, m * 128:(m + 1) * 128], wv_sb[:, k, :],
                start=(k == 0), stop=(k == 7))
        nc.vector.tensor_copy(
            out=V_sb[:, m].rearrange("p (h c) -> p h c", c=65)[:, :, 0:64],
            in_=ps[:, :256].rearrange("p (h c) -> p h c", c=64))

    attnT_sb = cst.tile([128, 2, S], BF16)

    def attend_qt(h, qt):
        sub, po = h // 2, 64 * (h % 2)
        at = pw.tile([128, 512], F32, tag="w", name=f"at{h}_{qt}")
        nkb = 4 * qt + 4
        for g0 in range(0, nkb, 2):
            gl = list(range(g0, min(g0 + 2, nkb)))
            sc = psc.tile([128, 1024], F32, tag="sc")
            for i, kb in enumerate(gl):
                rel = max(0, kb * 128 - qt * 512)
                nc.tensor.matmul(
                    sc[:, i * 512:(i + 1) * 512],
                    qkT_sb[po:po + 64, 2 + sub, kb * 128:(kb + 1) * 128],
                    qkT_sb[po:po + 64, sub, qt * 512:(qt + 1) * 512],
                    start=True, stop=True)
                if kb * 128 >= qt * 512:  # diagonal 128x128 triangle mask
                    nc.vector.tensor_tensor(
                        sc[:, i * 512 + rel:i * 512 + rel + 128],
                        sc[:, i * 512 + rel:i * 512 + rel + 128],
                        cm_sb[:], ADD)
            pt = sb.tile([128, 1024], BF16, tag="pt")
            w = len(gl) * 512
            nc.scalar.activation(out=pt[:, :w], in_=sc[:, :w], func=EXP)
            for i, kb in enumerate(gl):
                rel = max(0, kb * 128 - qt * 512)
                nc.tensor.matmul(
                    at[0:65, rel:512], V_sb[:, kb, h * 65:(h + 1) * 65],
                    pt[:, i * 512 + rel:(i + 1) * 512],
                    start=(kb == 0), stop=(kb == nkb - 1))
        # stash unnormalized attn, take 1/denominator, broadcast, normalize
        sl = attnT_sb[po:po + 64, sub, qt * 512:(qt + 1) * 512]
        nc.vector.tensor_copy(out=sl, in_=at[0:64, :])
        den1 = sb.tile([1, 512], F32, tag="den1")
        nc.vector.tensor_copy(out=den1[:], in_=at[64:65, :])
        rec1 = sb.tile([1, 512], F32, tag="rec1")
        nc.vector.reciprocal_approx_fast(rec1[:], den1[:])
        rec1b = sb.tile([1, 512], BF16, tag="rec1b")
        nc.vector.tensor_copy(out=rec1b[:], in_=rec1[:])
        bc = pw.tile([128, 512], F32, tag="w", name=f"bc{h}_{qt}")
        nc.tensor.matmul(
            bc[0:64, :], ones_sb[:], rec1b[:], start=True, stop=True)
        nc.vector.tensor_tensor(sl, sl, bc[0:64, :], MULT)

    def c_proj(ms):
        # partial c_proj: contract only this core's 256 D-rows (2 u-blocks of
        # 128), full 2048-seq output; host sums partials across head groups
        for m in ms:
            out_sb = sb.tile([128, D], F16, tag="out")
            ps = [pw.tile([128, 512], F32, tag="w", name=f"pj{m}_{n}") for n in range(2)]
            for u in range(2):
                for n in range(2):
                    nc.tensor.matmul(
                        ps[n][:], attnT_sb[:, u, m * 128:(m + 1) * 128],
                        wp_sb[:, u, n * 512:(n + 1) * 512],
                        start=(u == 0), stop=(u == 1))
            for n in range(2):
                nc.vector.tensor_copy(
                    out=out_sb[:, n * 512:(n + 1) * 512], in_=ps[n][:])
            nc.sync.dma_start(out[m * 128:(m + 1) * 128, :], out_sb[:])

    # ---- emission order tuned for overlap ----
    # attention rounds per qt with c_proj blocks right after each round so the
    # PE stream stays dense while ACT chews exps
    qk_proj(0)
    qk_proj(2)
    v_ones()
    for m in range(4):
        v_piece(m)
    attend_qt(0, 0)
    attend_qt(1, 0)
    qk_proj(1)
    qk_proj(3)
    for m in range(4, 16):
        v_piece(m)
    attend_qt(2, 0)
    attend_qt(3, 0)
    c_proj((0, 1, 2, 3))
    for qt in range(1, 4):
        for h in range(4):
            attend_qt(h, qt)
        c_proj(tuple(range(4 * qt, 4 * qt + 4)))

    ctx.close()


def build_nc():
    nc = bacc.Bacc("TRN2", target_bir_lowering=False, debug=False, num_devices=NCORES)
    with tile.TileContext(nc) as tc:
        _emit(nc, tc)
    nc.compile()
    return nc


def shard_inputs(hidden_states, c_attn_w, c_attn_b, c_proj_w, c_proj_b):
    x = np.asarray(hidden_states, np.float32)
    W = np.asarray(c_attn_w, np.float32)
    bqkv = np.asarray(c_attn_b, np.float32)
    Wp = np.asarray(c_proj_w, np.float32)

    wq, wk, wv = W[:, :D] * 0.125, W[:, D:2 * D], W[:, 2 * D:]
    bq, bk = bqkv[:D] * 0.125, bqkv[D:2 * D]

    # 128x128 causal triangle: -1e4 where key (row) > query (col)
    k_i = np.arange(128)[:, None]
    q_i = np.arange(128)[None, :]
    cm = np.where(k_i > q_i, np.float32(-10000.0), np.float32(0.0)).astype(np.float32)

    in_maps = []
    for c in range(NCORES):
        b, r = divmod(c, 4)
        hs = slice(256 * r, 256 * (r + 1))
        w_qk = np.concatenate([wq[:, hs], wk[:, hs]], axis=1)
        bqk_t = np.concatenate([bq[hs], bk[hs]]).reshape(4, 128).T.copy()
        in_maps.append(dict(
            xT=np.ascontiguousarray(x[b].T).astype(ml_dtypes.bfloat16),
            w_qk=w_qk.astype(ml_dtypes.bfloat16),
            w_v=wv[:, hs].astype(ml_dtypes.bfloat16),
            w_p=np.ascontiguousarray(Wp[hs, :]).astype(ml_dtypes.bfloat16),
            bqk=bqk_t.astype(np.float32),
            cmask=cm,
        ))
    return in_maps


def unshard(results, c_attn_b, c_proj_w, c_proj_b):
    bqkv = np.asarray(c_attn_b, np.float32)
    Wp = np.asarray(c_proj_w, np.float32)
    bp = np.asarray(c_proj_b, np.float32)
    # softmax rows sum to 1, so the v-bias passes through attention unchanged:
    # out = (softmax @ xWv + bv) @ Wp + bp = sum(partials) + bv@Wp + bp
    beff = (bqkv[2 * D:] @ Wp + bp).astype(np.float32)
    full = np.zeros((B, S, D), np.float32)
    for c in range(NCORES):
        b = c // 4
        full[b] += results[c]["out"].astype(np.float32)
    full += beff
    return full


_NC = None


def kernel(**inputs):
    global _NC
    if _NC is None:
        _NC = build_nc()
    in_maps = shard_inputs(**inputs)
    res = run_bass_kernel_spmd(_NC, in_maps, core_ids=list(range(NCORES)))
    return unshard(res.results, inputs["c_attn_b"], inputs["c_proj_w"],
                   inputs["c_proj_b"])


if __name__ == "__main__":
    import jax
    with jax.default_device(jax.devices("cpu")[0]):
        import reference
        inputs = {k: np.asarray(v) for k, v in reference.setup_inputs().items()}
        expected = np.asarray(reference.reference(**inputs))
    actual = kernel(**inputs)
    err = np.abs(actual - expected)
    print("max abs err:", err.max(), "rel:", err.max() / np.abs(expected).max())


# revision 5
# speedup vs baseline: 1.9360x; 1.3011x over previous
"""Trainium2 Bass kernel for GPT-2 style attention block (B=2, S=2048, D=1024, H=16).

Sharding (8 cores): data-parallel over batch (2) x tensor-parallel over heads (4 per
core). Each core: QKV projection for its 4 heads over the full sequence, full-seq
causal attention (transposed-scores layout: softmax reduction folded into the PV
matmul via a ones-column in V), then a row-parallel partial c_proj over the full
sequence using only this core's 256 rows of c_proj_w. No collectives: the host
sums the 4 per-head-group partials per batch (plus the folded v-bias term), so
each core's span is pure compute with no cross-core sync.

Schedule: work is organized in per-qt rounds (512 query columns each). The q/k
projections for query-block qt and key-block qt plus the V pieces for that key
range are emitted inside the round, so the tensor engine always has dense matmul
work to overlap the softmax exps (scalar engine) and stays HAM-warm. Softmax
normalization is off-PE: reciprocal on DVE straight from PSUM, partition
broadcast on GpSimd, fused scale-multiply on DVE.

Compute dtype bf16 (fp32 PSUM accumulation); masks/normalization in fp32;
partial outputs shipped as fp16 to halve DMA.
"""
import sys
sys.path.insert(0, '/opt/trn_rl_repo')

import numpy as np
import ml_dtypes

import concourse.bass as bass
import concourse.mybir as mybir
import concourse.tile as tile
from concourse import bacc
from concourse.bass_utils import run_bass_kernel_spmd

B, S, D = 2, 2048, 1024
H, HD = 16, 64
NCORES = 8
HPC = H // 4          # heads per core = 4

F32 = mybir.dt.float32
F16 = mybir.dt.float16
BF16 = mybir.dt.bfloat16
ADD = mybir.AluOpType.add
MULT = mybir.AluOpType.mult
EXP = mybir.ActivationFunctionType.Exp


def _emit(nc, tc):
    xT = nc.dram_tensor("xT", [D, S], BF16, kind="ExternalInput").ap()
    w_qk = nc.dram_tensor("w_qk", [D, 512], BF16, kind="ExternalInput").ap()
    w_v = nc.dram_tensor("w_v", [D, 256], BF16, kind="ExternalInput").ap()
    w_p = nc.dram_tensor("w_p", [256, D], BF16, kind="ExternalInput").ap()
    bqk = nc.dram_tensor("bqk", [128, 4], F32, kind="ExternalInput").ap()
    cmask = nc.dram_tensor("cmask", [128, 128], F32, kind="ExternalInput").ap()
    out = nc.dram_tensor("out", [S, D], F16, kind="ExternalOutput").ap()

    from contextlib import ExitStack
    ctx = ExitStack()
    cst = ctx.enter_context(tc.tile_pool(name="cst", bufs=1))
    pw = ctx.enter_context(tc.tile_pool(name="pw", bufs=2, space="PSUM"))
    pat = ctx.enter_context(tc.tile_pool(name="pat", bufs=2, space="PSUM"))
    psc = ctx.enter_context(tc.tile_pool(name="psc", bufs=2, space="PSUM"))
    sb = ctx.enter_context(tc.tile_pool(name="sb", bufs=3))

    # ---- resident SBUF loads (split per k-subtile so PE can start early) ----
    xT_sb = cst.tile([128, 8, S], BF16)
    wqk_sb = cst.tile([128, 8, 512], BF16)
    wv_sb = cst.tile([128, 8, 256], BF16)
    for k in range(8):
        nc.sync.dma_start(xT_sb[:, k], xT.rearrange("(k p) n -> p k n", p=128)[:, k])
        nc.sync.dma_start(wqk_sb[:, k], w_qk.rearrange("(k p) n -> p k n", p=128)[:, k])
        nc.sync.dma_start(wv_sb[:, k], w_v.rearrange("(k p) n -> p k n", p=128)[:, k])
    wp_sb = cst.tile([128, 2, D], BF16)
    nc.sync.dma_start(wp_sb[:], w_p.rearrange("(k p) n -> p k n", p=128))
    bqk_sb = cst.tile([128, 4], F32)
    nc.sync.dma_start(bqk_sb[:], bqk)
    cm_sb = cst.tile([128, 128], F32)
    nc.sync.dma_start(cm_sb[:], cmask)

    # PE warmer: dependency-free junk matmuls keep the array busy during the
    # input DMAs so HAM unthrottles before real work arrives
    ones_sb = cst.tile([1, 64], BF16)
    nc.vector.memset(ones_sb[:], 1.0)
    wrow = sb.tile([1, 512], BF16, tag="wrow")
    nc.vector.memset(wrow[:], 1.0)
    warm_ps = pw.tile([128, 512], F32, tag="w", name="warm")
    for _ in range(40):
        nc.tensor.matmul(warm_ps[0:64, :], ones_sb[:], wrow[:],
                         start=True, stop=True)

    # qkT [512, 2048]: rows 0-255 = q^T (4 heads x 64, prescaled 1/8), 256-511 = k^T
    qkT_sb = cst.tile([128, 4, S], BF16)

    def qk_proj(m, qt):
        # q^T (m=0,1) / k^T (m=2,3) for one 512-column sequence block
        ps = pw.tile([128, 512], F32, tag="w", name=f"qk{m}_{qt}")
        for k in range(8):
            nc.tensor.matmul(
                ps[:], wqk_sb[:, k, m * 128:(m + 1) * 128],
                xT_sb[:, k, qt * 512:(qt + 1) * 512],
                start=(k == 0), stop=(k == 7))
        nc.vector.tensor_scalar(
            out=qkT_sb[:, m, qt * 512:(qt + 1) * 512], in0=ps[:],
            scalar1=bqk_sb[:, m:m + 1], scalar2=None, op0=ADD)

    # V with interleaved ones column: V_sb [128, 16, 4*65]
    V_sb = cst.tile([128, 16, HPC * 65], BF16)

    def v_ones():
        nc.vector.memset(
            V_sb[:].rearrange("p m (h c) -> p m h c", c=65)[:, :, :, 64:65], 1.0)

    def v_piece(m):
        ps = pw.tile([128, 512], F32, tag="w", name=f"v{m}")
        for k in range(8):
            nc.tensor.matmul(
                ps[:, :256], xT_sb[:, k, m * 128:(m + 1) * 128], wv_sb[:, k, :],
                start=(k == 0), stop=(k == 7))
        nc.vector.tensor_copy(
            out=V_sb[:, m].rearrange("p (h c) -> p h c", c=65)[:, :, 0:64],
            in_=ps[:, :256].rearrange("p (h c) -> p h c", c=64))

    attnT_sb = cst.tile([128, 2, S], BF16)

    def attend_qt(h, qt):
        sub, po = h // 2, 64 * (h % 2)
        at = pat.tile([128, 512], F32, tag="at", name=f"at{h}_{qt}")
        nkb = 4 * qt + 4
        for g0 in range(0, nkb, 2):
            gl = list(range(g0, min(g0 + 2, nkb)))
            sc = psc.tile([128, 1024], F32, tag="sc")
            for i, kb in enumerate(gl):
                rel = max(0, kb * 128 - qt * 512)
                nc.tensor.matmul(
                    sc[:, i * 512 + rel:(i + 1) * 512],
                    qkT_sb[po:po + 64, 2 + sub, kb * 128:(kb + 1) * 128],
                    qkT_sb[po:po + 64, sub, qt * 512 + rel:(qt + 1) * 512],
                    start=True, stop=True)
                if kb * 128 >= qt * 512:  # diagonal 128x128 triangle mask
                    nc.vector.tensor_tensor(
                        sc[:, i * 512 + rel:i * 512 + rel + 128],
                        sc[:, i * 512 + rel:i * 512 + rel + 128],
                        cm_sb[:], ADD)
            pt = sb.tile([128, 1024], BF16, tag="pt")
            w = len(gl) * 512
            nc.scalar.activation(out=pt[:, :w], in_=sc[:, :w], func=EXP)
            for i, kb in enumerate(gl):
                rel = max(0, kb * 128 - qt * 512)
                nc.tensor.matmul(
                    at[0:65, rel:512], V_sb[:, kb, h * 65:(h + 1) * 65],
                    pt[:, i * 512 + rel:(i + 1) * 512],
                    start=(kb == 0), stop=(kb == nkb - 1))
        # normalize off-PE: denominator to SBUF, 1/x (DVE), broadcast partition
        # 0 -> 64 (GpSimd), fused scale-multiply PSUM->SBUF (DVE)
        den1 = sb.tile([1, 512], F32, tag="den1")
        nc.vector.tensor_copy(out=den1[:], in_=at[64:65, :])
        rec1 = sb.tile([1, 512], F32, tag="rec1")
        nc.vector.reciprocal_approx_fast(rec1[:], den1[:])
        recb = sb.tile([64, 512], F32, tag="recb")
        nc.gpsimd.partition_broadcast(recb[:], rec1[:])
        sl = attnT_sb[po:po + 64, sub, qt * 512:(qt + 1) * 512]
        nc.vector.tensor_tensor(sl, at[0:64, :], recb[:], MULT)

    def c_proj(ms):
        # partial c_proj: contract only this core's 256 D-rows (2 u-blocks of
        # 128), full 2048-seq output; host sums partials across head groups
        for m in ms:
            out_sb = sb.tile([128, D], F16, tag="out")
            ps = [pw.tile([128, 512], F32, tag="w", name=f"pj{m}_{n}") for n in range(2)]
            for u in range(2):
                for n in range(2):
                    nc.tensor.matmul(
                        ps[n][:], attnT_sb[:, u, m * 128:(m + 1) * 128],
                        wp_sb[:, u, n * 512:(n + 1) * 512],
                        start=(u == 0), stop=(u == 1))
            for n in range(2):
                nc.vector.tensor_copy(
                    out=out_sb[:, n * 512:(n + 1) * 512], in_=ps[n][:])
            nc.sync.dma_start(out[m * 128:(m + 1) * 128, :], out_sb[:])

    # ---- per-qt rounds: just-in-time projections keep the PE stream dense ----
    v_ones()
    for qt in range(4):
        for m in range(4):
            qk_proj(m, qt)          # q-block qt + k-block qt for all 4 heads
        for m in range(4 * qt, 4 * qt + 4):
            v_piece(m)              # V for key blocks of this round
        for h in range(4):
            attend_qt(h, qt)
        if qt:
            c_proj(tuple(range(4 * (qt - 1), 4 * qt)))
    c_proj(tuple(range(12, 16)))

    ctx.close()


def build_nc():
    nc = bacc.Bacc("TRN2", target_bir_lowering=False, debug=False, num_devices=NCORES)
    with tile.TileContext(nc) as tc:
        _emit(nc, tc)
    nc.compile()
    return nc


def shard_inputs(hidden_states, c_attn_w, c_attn_b, c_proj_w, c_proj_b):
    x = np.asarray(hidden_states, np.float32)
    W = np.asarray(c_attn_w, np.float32)
    bqkv = np.asarray(c_attn_b, np.float32)
    Wp = np.asarray(c_proj_w, np.float32)

    wq, wk, wv = W[:, :D] * 0.125, W[:, D:2 * D], W[:, 2 * D:]
    bq, bk = bqkv[:D] * 0.125, bqkv[D:2 * D]

    # 128x128 causal triangle: -1e4 where key (row) > query (col)
    k_i = np.arange(128)[:, None]
    q_i = np.arange(128)[None, :]
    cm = np.where(k_i > q_i, np.float32(-10000.0), np.float32(0.0)).astype(np.float32)

    in_maps = []
    for c in range(NCORES):
        b, r = divmod(c, 4)
        hs = slice(256 * r, 256 * (r + 1))
        w_qk = np.concatenate([wq[:, hs], wk[:, hs]], axis=1)
        bqk_t = np.concatenate([bq[hs], bk[hs]]).reshape(4, 128).T.copy()
        in_maps.append(dict(
            xT=np.ascontiguousarray(x[b].T).astype(ml_dtypes.bfloat16),
            w_qk=w_qk.astype(ml_dtypes.bfloat16),
            w_v=wv[:, hs].astype(ml_dtypes.bfloat16),
            w_p=np.ascontiguousarray(Wp[hs, :]).astype(ml_dtypes.bfloat16),
            bqk=bqk_t.astype(np.float32),
            cmask=cm,
        ))
    return in_maps


def unshard(results, c_attn_b, c_proj_w, c_proj_b):
    bqkv = np.asarray(c_attn_b, np.float32)
    Wp = np.asarray(c_proj_w, np.float32)
    bp = np.asarray(c_proj_b, np.float32)
    # softmax rows sum to 1, so the v-bias passes through attention unchanged:
    # out = (softmax @ xWv + bv) @ Wp + bp = sum(partials) + bv@Wp + bp
    beff = (bqkv[2 * D:] @ Wp + bp).astype(np.float32)
    full = np.zeros((B, S, D), np.float32)
    for c in range(NCORES):
        b = c // 4
        full[b] += results[c]["out"].astype(np.float32)
    full += beff
    return full


_NC = None


def kernel(**inputs):
    global _NC
    if _NC is None:
        _NC = build_nc()
    in_maps = shard_inputs(**inputs)
    res = run_bass_kernel_spmd(_NC, in_maps, core_ids=list(range(NCORES)))
    return unshard(res.results, inputs["c_attn_b"], inputs["c_proj_w"],
                   inputs["c_proj_b"])


if __name__ == "__main__":
    import jax
    with jax.default_device(jax.devices("cpu")[0]):
        import reference
        inputs = {k: np.asarray(v) for k, v in reference.setup_inputs().items()}
        expected = np.asarray(reference.reference(**inputs))
    actual = kernel(**inputs)
    err = np.abs(actual - expected)
    print("max abs err:", err.max(), "rel:", err.max() / np.abs(expected).max())
